# revision 20
# baseline (speedup 1.0000x reference)
"""GAT kernel for trn2, 8-core SPMD.

Math: nodes = x.transpose(2,0,1,3).reshape(63, 256000); h = nodes @ W;
a_src = h@att_src; a_dst = h@att_dst; e = leaky(a_dst[:,None]+a_src[None,:]);
out = softmax(e,1) @ h + bias, then mean over channels -> (63,1).

Since out.mean(1) = softmax(e) @ (h.mean(1)) + bias.mean(), the full h is
never needed: only three linear functionals of h (a_src, a_dst, hbar).
Each core contracts its shard of (x, W) into a partial (63,3); one 756-byte
AllReduce combines them; the 63x63 softmax epilogue runs on every core.

Layout trick: per-core W rows are i = a*250 + d (a = local batch index,
d = time sample), so the W shard viewed as [a=128 part, d=250, o=256] and
the x shard as [a=128 part, c=63, d=250] are BOTH fully contiguous per
partition -> max-efficiency DMA, no on-chip transposes of x.  The PE then
contracts over a: for each d, stationary lhsT = x[:, :, d] (128x63, 63-col
weight load) while W[:, d, :] streams 256 columns into PSUM h[63, 256].
"""

import numpy as np

A, B, C, D = 1024, 1, 63, 250
IN_CH = A * B * D
OUT_CH = 256
NEG_SLOPE = 0.2
N_CORES = 8
A_PER_CORE = A // N_CORES          # 128
ROWS_PER_CORE = A_PER_CORE * D     # 32000
DCH = 50                           # d-samples per W DMA chunk (25.6KB/partition bf16)
NCH = D // DCH                     # 10

CC_KIND = "AllReduce"  # AllGather variant measured worse on HW (63.9 vs 53.9 us)
W_BUFS = 3          # W chunk triple buffering
H_BUFS = 2          # h PSUM accumulator double-buffer across reps
EPI_RING = "sync"   # ring for small epilogue DMAs
X_RING = "scalar"   # ring for the x load (parallel to W on sync)
W_RING_ALT = False  # alternate W chunks across both HWDGE rings
W_SCALAR_CHUNKS = ()  # W chunk indices routed to the scalar ring (with x)

_CACHE = {}
LAST_RESULT = None


def _build(repeat=1):
    import concourse.mybir as mybir
    import concourse.tile as tile
    from concourse import bacc
    from concourse.masks import make_identity

    f32 = mybir.dt.float32
    bf16 = mybir.dt.bfloat16
    X = mybir.AxisListType.X
    add = mybir.AluOpType.add
    bypass = mybir.AluOpType.bypass
    mult = mybir.AluOpType.mult
    amax = mybir.AluOpType.max

    nc = bacc.Bacc("TRN2", target_bir_lowering=False, debug=False,
                   num_devices=N_CORES)

    xs_d = nc.dram_tensor("xs", [A_PER_CORE, C, D], bf16, kind="ExternalInput")
    Ws_d = nc.dram_tensor("Ws", [ROWS_PER_CORE, OUT_CH], bf16, kind="ExternalInput")
    asrc_d = nc.dram_tensor("att_src", [2, 128], f32, kind="ExternalInput")
    adst_d = nc.dram_tensor("att_dst", [2, 128], f32, kind="ExternalInput")
    bias_d = nc.dram_tensor("bias", [1, OUT_CH], f32, kind="ExternalInput")
    out_d = nc.dram_tensor("out", [C, 1], f32, kind="ExternalOutput")
    cc_in = nc.dram_tensor("cc_in", [C, 3], f32)
    cc_rows = N_CORES * C if CC_KIND == "AllGather" else C
    cc_out = nc.dram_tensor("cc_out", [cc_rows, 3], f32, addr_space="Shared")

    Wv = Ws_d.rearrange("(a d) o -> a d o", d=D)   # [128, 250, 256]

    epi_eng = {"sync": nc.sync, "scalar": nc.scalar}[EPI_RING]
    x_eng = {"sync": nc.sync, "scalar": nc.scalar}[X_RING]

    with tile.TileContext(nc) as tc:
        with (
            tc.tile_pool(name="const", bufs=1) as constp,
            tc.tile_pool(name="w", bufs=W_BUFS) as wp,
            tc.tile_pool(name="x", bufs=2) as xp,
            tc.tile_pool(name="hps", bufs=H_BUFS, space="PSUM") as accp,
            tc.tile_pool(name="eps", bufs=1, space="PSUM") as epp,
            tc.tile_pool(name="ep", bufs=1) as ep,
        ):
            ident = constp.tile([128, 128], f32)
            make_identity(nc, ident[:, :])
            ones = constp.tile([1, C], f32)
            nc.vector.memset(ones[0:1, :], 1.0)

            # constants: P = [att_src | att_dst | 1/256] and mean(bias)
            P_sb = constp.tile([128, 2, 3], f32)
            nc.sync.dma_start(out=P_sb[:, :, 0],
                              in_=asrc_d[:, :].rearrange("c p -> p c"))
            nc.sync.dma_start(out=P_sb[:, :, 1],
                              in_=adst_d[:, :].rearrange("c p -> p c"))
            nc.vector.memset(P_sb[:, :, 2], 1.0 / OUT_CH)
            bt = constp.tile([1, OUT_CH], f32)
            nc.sync.dma_start(out=bt[0:1, :], in_=bias_d[:, :])
            bsum = constp.tile([1, 1], f32)
            nc.vector.reduce_sum(bsum[0:1, :], bt[0:1, :], axis=X)
            nc.vector.tensor_scalar_mul(bsum[0:1, :], bsum[0:1, :], 1.0 / OUT_CH)

            for _rep in range(repeat):
                # full x shard resident: [a, c, d], 31.5KB/partition contiguous
                xt = xp.tile([A_PER_CORE, C, D], bf16, tag="xt")
                x_eng.dma_start(out=xt[:, :, :], in_=xs_d[:, :, :])

                h_ps = accp.tile([C, OUT_CH], f32, tag="h")
                for ch in range(NCH):
                    d0 = ch * DCH
                    wt = wp.tile([A_PER_CORE, DCH, OUT_CH], bf16, tag="wt")
                    w_eng = nc.scalar if ((W_RING_ALT and ch % 2) or ch in W_SCALAR_CHUNKS) else nc.sync
                    w_eng.dma_start(out=wt[:, :, :],
                                    in_=Wv[:, d0:d0 + DCH, :])
                    for dd in range(DCH):
                        d = d0 + dd
                        nc.tensor.matmul(h_ps[:, :], xt[:, :, d], wt[:, dd, :],
                                         start=(d == 0), stop=(d == D - 1))

                # ---- epilogue: h (63,256) -> hT (128,2,63) -> (63,3) ----
                h_sb = ep.tile([C, OUT_CH], f32)
                nc.vector.tensor_copy(h_sb[:, :], h_ps[:, :])

                hT_ps = epp.tile([128, 2, C], f32, tag="hT")
                nc.tensor.transpose(hT_ps[:, 0, :], h_sb[:, 0:128],
                                    ident[0:C, 0:C])
                nc.tensor.transpose(hT_ps[:, 1, :], h_sb[:, 128:256],
                                    ident[0:C, 0:C])
                hTs = ep.tile([128, 2, C], f32)
                nc.vector.tensor_copy(hTs[:, :, :], hT_ps[:, :, :])

                acb_ps = epp.tile([C, 3], f32, tag="acb")
                for c2 in range(2):
                    nc.tensor.matmul(acb_ps[:, :], hTs[:, c2, :], P_sb[:, c2, :],
                                     start=c2 == 0, stop=c2 == 1)
                acb_sb = ep.tile([C, 3], f32)
                nc.vector.tensor_copy(acb_sb[:, :], acb_ps[:, :])
                epi_eng.dma_start(out=cc_in[:, :], in_=acb_sb[:, :])

                nc.gpsimd.collective_compute(
                    CC_KIND,
                    bypass if CC_KIND == "AllGather" else add,
                    replica_groups=[list(range(N_CORES))],
                    ins=[cc_in.ap()], outs=[cc_out.ap()],
                )

                acb = ep.tile([C, 3], f32)
                if CC_KIND == "AllGather":
                    acb_all = ep.tile([C, N_CORES, 3], f32)
                    epi_eng.dma_start(
                        out=acb_all[:, :, :],
                        in_=cc_out[:, :].rearrange("(k c) t -> c k t", c=C))
                    nc.vector.tensor_tensor(acb[:, :], acb_all[:, 0, :],
                                            acb_all[:, 1, :], add)
                    for kk in range(2, N_CORES):
                        nc.vector.tensor_tensor(acb[:, :], acb[:, :],
                                                acb_all[:, kk, :], add)
                else:
                    epi_eng.dma_start(out=acb[:, :], in_=cc_out[:, :])

                # rows: a_src and hbar as (1,63) rows via tiny PE transposes
                rows_ps = epp.tile([1, 126], f32, tag="rows")
                nc.tensor.transpose(rows_ps[0:1, 0:63], acb[:, 0:1],
                                    ident[0:C, 0:C])
                nc.tensor.transpose(rows_ps[0:1, 63:126], acb[:, 2:3],
                                    ident[0:C, 0:C])
                rows = ep.tile([1, 126], f32)
                nc.vector.tensor_copy(rows[0:1, :], rows_ps[0:1, :])

                # hbar' = hbar + mean(bias)
                nc.vector.tensor_scalar_add(rows[0:1, 63:126], rows[0:1, 63:126],
                                            bsum[0:1, :])

                # broadcast rows across the 63 node partitions via a K=1
                # outer-product matmul: bc[i, :] = rows[0, :] for all i
                bc_ps = epp.tile([C, 126], f32, tag="bc")
                nc.tensor.matmul(bc_ps[:, :], ones[0:1, :], rows[0:1, :],
                                 start=True, stop=True)
                asb = ep.tile([C, 126], f32)
                nc.vector.tensor_copy(asb[:, :], bc_ps[:, :])

                # e = leaky_relu(a_dst[i] + a_src[j])
                u = ep.tile([C, C], f32)
                nc.vector.tensor_scalar(u[:, :], asb[:, 0:63], acb[:, 1:2], None,
                                        add)
                u2 = ep.tile([C, C], f32)
                nc.vector.tensor_scalar_mul(u2[:, :], u[:, :], NEG_SLOPE)
                e = ep.tile([C, C], f32)
                nc.vector.tensor_tensor(e[:, :], u[:, :], u2[:, :], amax)

                # softmax-weighted sum of hbar'
                nm = ep.tile([C, 1], f32)
                nc.vector.reduce_max(nm[:, :], e[:, :], axis=X, negate=True)
                pexp = ep.tile([C, C], f32)
                s = ep.tile([C, 1], f32)
                nc.scalar.activation(pexp[:, :], e[:, :],
                                     mybir.ActivationFunctionType.Exp,
                                     bias=nm[:, :], scale=1.0, accum_out=s[:, :])
                prod = ep.tile([C, C], f32)
                tsum = ep.tile([C, 1], f32)
                nc.vector.tensor_tensor(prod[:, :], pexp[:, :], asb[:, 63:126],
                                        mult)
                nc.vector.reduce_sum(tsum[:, :], prod[:, :], axis=X)
                rs = ep.tile([C, 1], f32)
                nc.vector.reciprocal(rs[:, :], s[:, :])
                oc = ep.tile([C, 1], f32)
                nc.vector.tensor_tensor(oc[:, :], tsum[:, :], rs[:, :], mult)
                epi_eng.dma_start(out=out_d[:, :], in_=oc[:, :])

    nc.compile()
    return nc


def make_in_maps(x, W, att_src, att_dst, bias):
    import ml_dtypes
    x = np.asarray(x, dtype=np.float32).astype(ml_dtypes.bfloat16)
    W = np.asarray(W, dtype=np.float32).astype(ml_dtypes.bfloat16)
    att_src = np.asarray(att_src, dtype=np.float32).reshape(2, 128)
    att_dst = np.asarray(att_dst, dtype=np.float32).reshape(2, 128)
    bias = np.asarray(bias, dtype=np.float32).reshape(1, OUT_CH)

    in_maps = []
    for k in range(N_CORES):
        in_maps.append({
            "xs": np.ascontiguousarray(x[k * A_PER_CORE:(k + 1) * A_PER_CORE, 0]),
            "Ws": np.ascontiguousarray(W[k * ROWS_PER_CORE:(k + 1) * ROWS_PER_CORE]),
            "att_src": att_src,
            "att_dst": att_dst,
            "bias": bias,
        })
    return in_maps


def kernel(x, W, att_src, att_dst, bias, trace=False):
    global LAST_RESULT
    from concourse.bass_utils import run_bass_kernel_spmd

    if "nc" not in _CACHE:
        _CACHE["nc"] = _build()
    nc = _CACHE["nc"]

    in_maps = make_in_maps(x, W, att_src, att_dst, bias)
    res = run_bass_kernel_spmd(nc, in_maps, core_ids=list(range(N_CORES)),
                               trace=trace)
    LAST_RESULT = res
    return res.results[0]["out"]
